# revision 44
# baseline (speedup 1.0000x reference)
"""Trainium2 Bass kernel for nn_CAA_Stable (stable-diffusion style channel
self-attention block over 64x64 feature maps).

Reference computation per batch b (C=256 channels, N=64*64=4096 positions):
    q = scale*(Wq@x + bq)  [D=16, N]
    k = Wk@x + bk          [D, N]
    logits[n,m] = q[:,n].k[:,m];  w = softmax(logits, axis=m)
    y = gamma_clipped * (Wo @ ((Wv@x+bv) @ w^T) + bo) + x

Key design (v3, fp8 attention-value path, exp pipelined a strip ahead):
  * Wo folds into Wv: U = (Wo@Wv) @ x -- one 256x256 projection pass; the
    value bias becomes a constant channel bias cbias = Wo@bv + bo folded
    into the residual (xgb = x + g*cbias, computed on the idle Pool
    engine).
  * Attention-value matmul in fp8 DoubleRow (2x PE throughput): the exp
    tiles are f8e5m2 (max 57344 -- exp(logit-1) peaks ~3e3, and dt.float8e4
    here is the INF-style e4m3 whose max is only 240, which overflows);
    U is f8e4m3 scaled by 32 (|32U| < 240). Scales cancel in the softmax
    normalization (the denominator uses a 32-valued e5m2 ones stationary,
    also DoubleRow). Verified in numpy: rel err ~7e-4.
  * exp (the ACT-engine bottleneck, ~1us per [128,1024] tile) is computed
    ONE STRIP AHEAD into a double-banked SBUF f8 cache, so the AV/den
    matmul stream never waits on ACT and strip boundaries don't stall:
    while strip s's AV consumes bank s%2, qk+exp for strip s+1 fill bank
    (s+1)%2. Strip 0's exps interleave into the phase-0 projection loop.
  * QK^T contraction (K=16) uses tile_position row packing with q and k
    REPLICATED into both 16-row halves of each 32-row group (logits come
    out doubled; q is pre-scaled by scale/2). The replication is free:
    the projection stationary wqT/wkT is replicated 8x along its free
    dim, so the projection matmul directly emits replicated [128, QS]
    q/k. Bias tiles replicate via a tiny rep_id matmul.
  * PSUM: qk 2x[128,2,512] (4 banks) + ua [128,2,512] (2) + den 2x[128,512]
    (2); phase-0 scratch rotates through the den+ua banks (idle then).

Sharding: pure data-parallel over batch, one image per NeuronCore, no
collectives. kernel() takes FULL inputs, returns the FULL output.
"""

import numpy as np

B, C, HW, D = 8, 256, 4096, 16
P = 128
QS = 512              # q-strip width (one PSUM bank)
NSTRIP = HW // QS     # 8
NKC = HW // P         # 32 key chunks of 128
KGRP = 2              # key chunks per group (one qk tile / DoubleRow pair)
NGRP = NKC // KGRP    # 16
SCALE = float(D) ** -0.5
EXP_SHIFT = -1.0      # exp(logit + EXP_SHIFT); cancels in normalization
USCALE = 32.0         # U/ones scale; keeps e4m3 (max 240, inf-style) safe
# Schraudolph bit-trick exp on DVE (offloads the ACT bottleneck):
# bitsAsFloat(int32(A*x + B)) ~ exp(x), max rel err ~3% -- invisible under
# the 12.5% e5m2 quantization. B folds in EXP_SHIFT.
EXP_A = float(2 ** 23 / np.log(2))
EXP_B = 1064985000.0 + EXP_A * EXP_SHIFT
DVE_EXP_GROUPS = (5, 11)  # per-strip groups computed on DVE instead of ACT

_cache = {}


def _build(repeat=1, dbg=0):
    import ml_dtypes
    import concourse.bacc as bacc
    import concourse.mybir as mybir
    import concourse.tile as tile

    dt = mybir.dt
    AF = mybir.ActivationFunctionType
    ALU = mybir.AluOpType
    f32, bf16, f8 = dt.float32, dt.bfloat16, dt.float8e4
    f8e5 = dt.float8e5
    PM = mybir.MatmulPerfMode

    nc = bacc.Bacc("TRN2", target_bir_lowering=False, debug=False, num_devices=B)

    x_d = nc.dram_tensor("x", [C, HW], f32, kind="ExternalInput")
    wq_d = nc.dram_tensor("Wq", [D, C], f32, kind="ExternalInput")
    bq_d = nc.dram_tensor("bq", [D], f32, kind="ExternalInput")
    wk_d = nc.dram_tensor("Wk", [D, C], f32, kind="ExternalInput")
    bk_d = nc.dram_tensor("bk", [D], f32, kind="ExternalInput")
    wv_d = nc.dram_tensor("Wv", [C, C], f32, kind="ExternalInput")
    bv_d = nc.dram_tensor("bv", [C], f32, kind="ExternalInput")
    wo_d = nc.dram_tensor("Wo", [C, C], f32, kind="ExternalInput")
    bo_d = nc.dram_tensor("bo", [C], f32, kind="ExternalInput")
    g_d = nc.dram_tensor("gamma", [1], f32, kind="ExternalInput")
    y_d = nc.dram_tensor("y", [C, HW], f32, kind="ExternalOutput")

    id_d = nc.inline_tensor(np.eye(P, dtype=np.float32), name="ident_c")
    ones82_d = nc.inline_tensor(
        np.full((P, 2, P), USCALE, dtype=ml_dtypes.float8_e5m2), name="ones82_c"
    )
    # rep_id[r, o] = (o % 16 == r): replicates a [16,1] bias to [128,1] via PE
    rep_np = (np.arange(P)[None, :] % D == np.arange(D)[:, None]).astype(
        ml_dtypes.bfloat16
    )
    repid_d = nc.inline_tensor(rep_np, name="repid_c")
    ones1_d = nc.inline_tensor(
        np.ones((1, P), dtype=ml_dtypes.bfloat16), name="ones1_c"
    )

    x3 = x_d.ap().rearrange("(a p) n -> a p n", p=P)
    y3 = y_d.ap().rearrange("(a p) n -> a p n", p=P)
    wv3 = wv_d.ap().rearrange("(a p) c -> a p c", p=P)
    wo3 = wo_d.ap().rearrange("(a p) c -> a p c", p=P)
    bv2 = bv_d.ap().rearrange("(a p) -> a p", p=P)
    bo2 = bo_d.ap().rearrange("(a p) -> a p", p=P)

    with tile.TileContext(nc) as tc:
        with (
            tc.tile_pool(name="const", bufs=1) as constp,
            tc.tile_pool(name="xpool", bufs=1) as xpool,
            tc.tile_pool(name="wpool", bufs=1) as wpool,
            tc.tile_pool(name="big", bufs=1) as big,
            tc.tile_pool(name="srp", bufs=2) as srp,
            tc.tile_pool(name="ytp", bufs=2) as ytp,
            tc.tile_pool(name="i32p", bufs=2) as i32p,
            tc.tile_pool(name="qkps", bufs=2, space="PSUM") as qkps,
            tc.tile_pool(name="uaps", bufs=1, space="PSUM") as uaps,
            tc.tile_pool(name="dps", bufs=2, space="PSUM") as dps,
        ):
            # phase-0 scratch rotates through the den/ua banks (idle then)
            _pctr = [0]

            def ppsum(shape):
                _pctr[0] += 1
                pool = (dps, uaps)[_pctr[0] % 2]
                return pool.tile(shape, f32, tag="scr" if pool is uaps else "d",
                                 name="scr")

            ident = constp.tile([P, P], f32, tag="ident", name="ident")
            nc.sync.dma_start(ident[:], id_d.ap())
            rep_id = constp.tile([D, P], bf16, tag="rep_id", name="rep_id")
            nc.sync.dma_start(rep_id[:], repid_d.ap())
            # ones_1/ones_82 DMAs are deferred into rep 0's stream -- they
            # are first needed ~20us in and must not delay wq/x(0) on the
            # serial hwdge.
            ones_1 = constp.tile([1, P], bf16, tag="ones_1", name="ones_1")
            ones_82 = constp.tile([P, 2, P], f8e5, tag="ones_82", name="ones_82")
            exb = constp.tile([P, 1], f32, tag="exb", name="exb")
            nc.vector.memset(exb[:], EXP_SHIFT)
            # Trigger the exp table-set load off the critical path (input is
            # the memset tile: no DMA dependency).
            warm = constp.tile([1, 1], f32, tag="warm", name="warm")
            nc.scalar.activation(warm[:], exb[:1, :1], AF.Exp)

            for rep in range(repeat):
                # ---------- phase 0: loads, weight prep, projections.
                # DMA emission order paces the (serial) hwdge: wq/wk/bq then
                # x(0) so the q/k pipeline starts ~3us in; wv/wo mid-stream
                # (first needed by the U chunks, which trail 2 strips); the
                # remaining bias vectors last.
                xs, xb = [], []
                for ci in range(2):
                    t = xpool.tile([P, HW], f32, tag=f"x{ci}", name=f"x{ci}")
                    tb = xpool.tile([P, HW], bf16, tag=f"xb{ci}", name=f"xb{ci}")
                    xs.append(t)
                    xb.append(tb)

                def xdma(s):
                    sl = slice(s * QS, (s + 1) * QS)
                    nc.sync.dma_start(xs[0][:, sl], x3[0][:, sl])
                    nc.sync.dma_start(xs[1][:, sl], x3[1][:, sl])

                xdma(0)
                wq_sb = wpool.tile([D, C], f32, tag="wq", name="wq")
                nc.sync.dma_start(wq_sb[:], wq_d.ap())
                wk_sb = wpool.tile([D, C], f32, tag="wk", name="wk")
                nc.sync.dma_start(wk_sb[:], wk_d.ap())
                with nc.allow_non_contiguous_dma(reason="tiny bias vectors"):
                    bq_sb = wpool.tile([D, 1], f32, tag="bq", name="bq")
                    nc.sync.dma_start(bq_sb[:], bq_d.ap()[:, None])
                xdma(1)
                wv_sb, wo_sb = [], []
                for e in range(2):
                    t = wpool.tile([P, C], f32, tag=f"wv{e}", name=f"wv{e}")
                    nc.sync.dma_start(t[:], wv3[e])
                    wv_sb.append(t)
                    t = wpool.tile([P, C], f32, tag=f"wo{e}", name=f"wo{e}")
                    nc.sync.dma_start(t[:], wo3[e])
                    wo_sb.append(t)
                    xdma(2 + e)
                if rep == 0:
                    nc.sync.dma_start(ones_82[:], ones82_d.ap())
                    nc.sync.dma_start(ones_1[:], ones1_d.ap())
                for s in range(4, NSTRIP):
                    xdma(s)
                bv_sb, bo_sb = [], []
                with nc.allow_non_contiguous_dma(reason="tiny bias vectors"):
                    for e in range(2):
                        t = wpool.tile([P, 1], f32, tag=f"bv{e}", name=f"bv{e}")
                        nc.sync.dma_start(t[:], bv2[e][:, None])
                        bv_sb.append(t)
                        t = wpool.tile([P, 1], f32, tag=f"bo{e}", name=f"bo{e}")
                        nc.sync.dma_start(t[:], bo2[e][:, None])
                        bo_sb.append(t)
                    g_sb = wpool.tile([1, 1], f32, tag="g", name="g")
                    nc.sync.dma_start(g_sb[:], g_d.ap()[:, None])

                # transposed q/k weights, replicated 8x along the free dim so
                # the projection matmul directly emits [128, QS] replicated
                # q/k (stationary [c, o] with col o holding Wq[o % 16, c]).
                # The k bias is dropped entirely: a per-query constant shift
                # of the logits cancels in softmax, and q_n.bk is constant
                # over keys, so only the q bias survives.
                wqT, wkT = [], []
                for ci in range(2):
                    for pre, dst, wsb, scl in (
                        ("q", wqT, wq_sb, SCALE / 2), ("k", wkT, wk_sb, 1.0)
                    ):
                        ps = ppsum([P, P])
                        nc.tensor.transpose(
                            ps[:, :D], wsb[:, ci * P : (ci + 1) * P], ident[:D, :D]
                        )
                        t = wpool.tile(
                            [P, P // D, D], bf16, tag=f"w{pre}T{ci}",
                            name=f"w{pre}T{ci}",
                        )
                        nc.vector.tensor_scalar(
                            t[:],
                            ps[:, None, :D].to_broadcast((P, P // D, D)),
                            scl, 0.0, ALU.mult, ALU.add,
                        )
                        dst.append(t)

                bqs = wpool.tile([D, 1], bf16, tag="bqs", name="bqs")
                nc.vector.tensor_scalar(
                    bqs[:], bq_sb[:], SCALE / 2, 0.0, ALU.mult, ALU.add
                )
                ps = ppsum([P, 1])
                nc.tensor.matmul(ps[:], rep_id[:], bqs[:], start=True, stop=True)
                bqs_rep = wpool.tile([P, 1], f32, tag="bqr", name="bqr")
                nc.vector.tensor_copy(bqs_rep[:], ps[:])

                # phase-1 tiles: q/k, U chunks, double-banked exp cache
                q_rep = big.tile([P, HW], bf16, tag="q_rep", name="q_rep")
                k_rep = big.tile([P, HW], bf16, tag="k_rep", name="k_rep")
                ut = big.tile([P, NKC, C], f8, tag="ut", name="ut")
                exbank = [
                    big.tile([P, NGRP, KGRP, QS], f8e5, tag=f"exb{i}", name=f"exb{i}")
                    for i in range(2)
                ]

                def emit_qk(s, g):
                    qk = qkps.tile([P, KGRP, QS], f32, tag="qk", name="qk")
                    for j in range(KGRP):
                        kc = KGRP * g + j
                        roff = 32 * (kc % 4)
                        nc.tensor.matmul(
                            qk[:, j, :],
                            k_rep[roff : roff + 32, kc * P : (kc + 1) * P],
                            q_rep[roff : roff + 32, s * QS : (s + 1) * QS],
                            start=True,
                            stop=True,
                            tile_position=(roff, 0),
                        )
                    return qk

                def emit_exp(s, g, qk, dve=False):
                    if dve:
                        t = i32p.tile([P, KGRP, QS], dt.int32, tag="i32",
                                      name="i32")
                        nc.vector.tensor_scalar(
                            t[:], qk[:], EXP_A, EXP_B, ALU.mult, ALU.add
                        )
                        nc.vector.tensor_copy(
                            exbank[s % 2][:, g], t[:].bitcast(f32)
                        )
                    else:
                        nc.scalar.activation(
                            exbank[s % 2][:, g], qk[:], AF.Exp, bias=exb[:]
                        )

                def emit_u(s, in_phase1=False):
                    # U chunks for strip s (trails 2 strips behind q/k so the
                    # wv/wo prep can ride mid-DMA-stream). In phase 1 the
                    # scratch must come from the qk rotation -- ua/den banks
                    # are live across the whole strip.
                    for pair in range(2):
                        kc0 = 4 * s + 2 * pair
                        ups = (
                            qkps.tile([P, 2, C], f32, tag="qk", name="scr")
                            if in_phase1 else ppsum([P, 2, C])
                        )
                        for off in range(2):
                            for ci in range(2):
                                nc.tensor.matmul(
                                    ups[:, off, :],
                                    xb[ci][:, (kc0 + off) * P : (kc0 + off + 1) * P],
                                    w2T[ci][:],
                                    start=(ci == 0), stop=(ci == 1),
                                )
                        # psum->f8 scale on DVE: ACT is the steady-state
                        # bottleneck (exp), keep everything else off it
                        nc.vector.tensor_scalar(
                            ut[:, kc0 : kc0 + 2, :], ups[:], USCALE, 0.0,
                            ALU.mult, ALU.add,
                        )

                # per-strip x casts + q/k projections, paced by the arriving
                # x strips; strip-0 qk+exp interleave here, lagging ONE strip
                # behind the k chunks they consume so the PE never stalls on
                # the k psum->sbuf copy
                woT = w2T = None
                for s in range(NSTRIP):
                    sl = slice(s * QS, (s + 1) * QS)
                    for ci in range(2):
                        nc.vector.tensor_copy(xb[ci][:, sl], xs[ci][:, sl])
                    ps = ppsum([P, QS])
                    for ci in range(2):
                        nc.tensor.matmul(
                            ps[:], wqT[ci][:], xb[ci][:, sl],
                            start=(ci == 0), stop=(ci == 1),
                        )
                    nc.vector.tensor_scalar_add(q_rep[:, sl], ps[:], bqs_rep[:])
                    ps = ppsum([P, QS])
                    for ci in range(2):
                        nc.tensor.matmul(
                            ps[:], wkT[ci][:], xb[ci][:, sl],
                            start=(ci == 0), stop=(ci == 1),
                        )
                    nc.vector.tensor_copy(k_rep[:, sl], ps[:])
                    if s >= 1:
                        # lagged strip-0 qk+exp: consume the PREVIOUS strip's
                        # q/k so the PE never stalls on the psum->sbuf copies
                        for gg in (2 * (s - 1), 2 * (s - 1) + 1):
                            emit_exp(0, gg, emit_qk(0, gg))
                    if s == 1:
                        # wo/wv prep (DMAs land mid-stream): woT, wvb, W2T
                        woT = [
                            wpool.tile([P, C], bf16, tag=f"woT{ei}", name=f"woT{ei}")
                            for ei in range(2)
                        ]
                        for ci in range(2):
                            for ei in range(2):
                                ps = ppsum([P, P])
                                nc.tensor.transpose(
                                    ps[:], wo_sb[ci][:, ei * P : (ei + 1) * P],
                                    ident[:],
                                )
                                nc.vector.tensor_copy(
                                    woT[ei][:, ci * P : (ci + 1) * P], ps[:]
                                )
                        wvb = []
                        for ei in range(2):
                            t = wpool.tile(
                                [P, C], bf16, tag=f"wvb{ei}", name=f"wvb{ei}"
                            )
                            nc.vector.tensor_copy(t[:], wv_sb[ei][:])
                            wvb.append(t)
                        w2T = []
                        for ci in range(2):
                            ps = ppsum([P, C])
                            for ei in range(2):
                                nc.tensor.matmul(
                                    ps[:],
                                    wvb[ei][:, ci * P : (ci + 1) * P],
                                    woT[ei][:],
                                    start=(ei == 0), stop=(ei == 1),
                                )
                            t = wpool.tile(
                                [P, C], bf16, tag=f"w2T{ci}", name=f"w2T{ci}"
                            )
                            nc.vector.tensor_copy(t[:], ps[:])
                            w2T.append(t)
                    if s >= 2:
                        emit_u(s - 2)
                for gg in (2 * (NSTRIP - 1), 2 * (NSTRIP - 1) + 1):
                    emit_exp(0, gg, emit_qk(0, gg))

                # gamma: clip to [0,1], replicate via the ones stationary
                nc.vector.tensor_scalar(g_sb[:], g_sb[:], 1.0, 0.0, ALU.min, ALU.max)
                gb = wpool.tile([1, 1], bf16, tag="gb", name="gb")
                nc.vector.tensor_copy(gb[:], g_sb[:])
                ps = ppsum([P, 1])
                nc.tensor.matmul(ps[:], ones_1[:], gb[:], start=True, stop=True)
                g_rep = wpool.tile([P, 1], f32, tag="grep", name="grep")
                nc.vector.tensor_copy(g_rep[:], ps[:])

                # gcb[fi] = g * (Wo@bv + bo) folded into the residual
                bvb = []
                for ei in range(2):
                    t = wpool.tile([P, 1], bf16, tag=f"bvb{ei}", name=f"bvb{ei}")
                    nc.vector.tensor_copy(t[:], bv_sb[ei][:])
                    bvb.append(t)
                gcb = []
                for fi in range(2):
                    ps = ppsum([P, 1])
                    for ei in range(2):
                        nc.tensor.matmul(
                            ps[:],
                            woT[ei][:, fi * P : (fi + 1) * P],
                            bvb[ei][:],
                            start=(ei == 0), stop=(ei == 1),
                        )
                    t = wpool.tile([P, 1], f32, tag=f"gcb{fi}", name=f"gcb{fi}")
                    nc.vector.tensor_scalar(
                        t[:], ps[:], bo_sb[fi][:], g_rep[:], ALU.add, ALU.mult
                    )
                    gcb.append(t)

                # xgb = x + g*cbias, in place over the f32 x tiles (on the
                # otherwise-idle Pool engine; only needed by the epilogues)
                for s in range(NSTRIP):
                    sl = slice(s * QS, (s + 1) * QS)
                    for fi in range(2):
                        nc.gpsimd.tensor_scalar_add(
                            xs[fi][:, sl], xs[fi][:, sl], gcb[fi][:]
                        )

                # ---------- phase 1: AV/den for strip s stream from the exp
                # bank while qk+exp for strip s+1 fill the other bank
                for s in range(NSTRIP):
                    sl = slice(s * QS, (s + 1) * QS)
                    ua = uaps.tile([P, 2, QS], f32, tag="scr", name="ua")
                    den = dps.tile([P, QS], f32, tag="d", name="d")
                    if s + 1 < NSTRIP:
                        # adjacent qk emissions cover 4 distinct 32-row
                        # groups, so the packed MMs overlap 4-way in the PE
                        pend = [(0, emit_qk(s + 1, 0)), (1, emit_qk(s + 1, 1))]
                    for g in range(NGRP):
                        exg = exbank[s % 2][:, g]
                        kc0 = KGRP * g
                        for fi in range(2):
                            nc.tensor.matmul(
                                ua[:, fi, :],
                                ut[:, kc0 : kc0 + 2, fi * P : (fi + 1) * P],
                                exg,
                                start=(g == 0),
                                stop=(g == NGRP - 1),
                                perf_mode=PM.DoubleRow,
                            )
                        nc.tensor.matmul(
                            den[:],
                            ones_82[:],
                            exg,
                            start=(g == 0),
                            stop=(g == NGRP - 1),
                            perf_mode=PM.DoubleRow,
                        )
                        if g % 2 == 1:
                            if s + 1 < NSTRIP:
                                for _ in range(2):
                                    gn, qk = pend.pop(0)
                                    emit_exp(s + 1, gn, qk,
                                             dve=gn in DVE_EXP_GROUPS)
                                for gg in (g + 1, g + 2):
                                    if gg < NGRP:
                                        pend.append((gg, emit_qk(s + 1, gg)))
                            if s == 0 and g == 1:
                                # trailing U chunks (strips 6,7) ride early in
                                # strip 0's AV stream, before AV needs them
                                emit_u(NSTRIP - 2, in_phase1=True)
                                emit_u(NSTRIP - 1, in_phase1=True)

                    # epilogue: y = ua * (g/den) + (x + g*cbias)
                    srep = srp.tile([P, QS], f32, tag="srep", name="srep")
                    nc.vector.reciprocal(srep[:], den[:])
                    nc.vector.tensor_scalar(
                        srep[:], srep[:], g_rep[:], 0.0, ALU.mult, ALU.add
                    )
                    for fi in range(2):
                        yt = ytp.tile([P, QS], f32, tag="yt", name="yt")
                        nc.vector.tensor_mul(yt[:], ua[:, fi, :], srep[:])
                        nc.vector.tensor_add(yt[:], yt[:], xs[fi][:, sl])
                        nc.sync.dma_start(y3[fi, :, sl], yt[:])

    nc.compile()
    return nc


def _get_nc(**kw):
    key = tuple(sorted(kw.items()))
    if key not in _cache:
        _cache[key] = _build(**kw)
    return _cache[key]


def _in_maps(inputs):
    names = ["Wq", "bq", "Wk", "bk", "Wv", "bv", "Wo", "bo", "gamma"]
    base = {
        n: np.ascontiguousarray(np.asarray(inputs[n], dtype=np.float32))
        for n in names
    }
    x = np.ascontiguousarray(np.asarray(inputs["x"], dtype=np.float32))
    assert x.shape == (B, C, 64, 64), x.shape
    maps = []
    for b in range(B):
        m = dict(base)
        m["x"] = np.ascontiguousarray(x[b].reshape(C, HW))
        maps.append(m)
    return maps


def _run(inputs, trace=False, build_kw=None, **kw):
    from concourse.bass_utils import run_bass_kernel_spmd

    nc = _get_nc(**(build_kw or {}))
    res = run_bass_kernel_spmd(
        nc, _in_maps(inputs), core_ids=list(range(B)), trace=trace, **kw
    )
    y = np.stack([r["y"] for r in res.results]).reshape(B, C, 64, 64)
    return np.ascontiguousarray(y.astype(np.float32)), res


def kernel(**inputs):
    y, _ = _run(inputs)
    return y
